# revision 71
# baseline (speedup 1.0000x reference)
"""Trainium2 Bass kernel for MinimalLightningIndexer.

out[b,t,s] = relu((x@Wq)[b,t] . (x@Wk)[b,s]) * (x@Ww)[b,t]

Sharding: 8 cores = 4 batches x 2 token-halves. Each core receives ONLY
its own half of x[b] (transposed, 8.4 MB bf16) and computes [k|q|w]
projections for its 2048 tokens in one fused PE pass. Phase A scores the
2048 own queries against the 2048 LOCAL keys immediately; the 16-dim key
rows are exchanged with the sibling core in two pipelined pairwise
AllGathers (2 x 32 KB), and phases B0/B1 score against the sibling keys
(selected from the gather output by a per-core index-gather DMA, keeping
the program SPMD-uniform). Output [2048, 4096] bf16 = [own | sibling]
columns; the host reassembles with 2 block copies per core.

PE clock note: score matmuls contract only 16 idx dims; with K=16 the
HAM activity monitor reads the array as idle and keeps the PE throttled
at 1.2 GHz. The stationary q operand is therefore zero-padded to K=128
(full-array activity + fast weight load), and warm-up matmuls keep the
array busy during the input load.
"""

import os
import sys

if "/opt/trn_rl_repo" not in sys.path:
    sys.path.insert(0, "/opt/trn_rl_repo")

import numpy as np

import concourse.bacc as bacc
import concourse.bass as bass
import concourse.mybir as mybir
import concourse.tile as tile
from concourse.bass_utils import run_bass_kernel_spmd

B, S, D = 4, 4096, 2048
IDX = 16
N_CORES = 8
T = S // 2           # own tokens per core
DC = D // 128        # 16 d-chunks
NG = 4               # projection groups (512 tokens each)
W33 = 2 * IDX + 1    # [k | q | w] projection width

USE_CC = os.environ.get("K_USE_CC", "1") == "1"    # AllGather k-exchange
FUSED = os.environ.get("K_FUSED", "1") == "1"      # DVE fused relu+gate
WARM_N = int(os.environ.get("K_WARM_N", "56"))     # PE warm-up matmuls

_CACHE = {}


def _build_nc():
    if "nc" in _CACHE:
        return _CACHE["nc"]
    f32 = mybir.dt.float32
    bf16 = mybir.dt.bfloat16
    i32 = mybir.dt.int32
    nc = bacc.Bacc("TRN2", target_bir_lowering=False, debug=False,
                   num_devices=N_CORES)

    n_slabs = NG if USE_CC else 2 * NG
    xh = nc.dram_tensor("xh", [n_slabs * 512, 2048], bf16,
                        kind="ExternalInput").ap()
    wkqw = nc.dram_tensor("wkqw", [128, DC * W33], bf16,
                          kind="ExternalInput").ap()
    if USE_CC:
        sel = nc.dram_tensor("sel", [IDX, 1], i32, kind="ExternalInput").ap()
    o = nc.dram_tensor("o", [T, S], bf16, kind="ExternalOutput").ap()

    groups = [[2 * i, 2 * i + 1] for i in range(N_CORES // 2)]

    with tile.TileContext(nc) as tc:
        with (
            tc.tile_pool(name="const", bufs=1) as cpool,
            tc.tile_pool(name="slab", bufs=3) as slab_pool,
            tc.tile_pool(name="osb", bufs=8) as out_pool,
            tc.tile_pool(name="pj", bufs=2, space="PSUM") as pj_pool,
            tc.tile_pool(name="ps", bufs=2, space="PSUM") as ps_pool,
            tc.tile_pool(name="pw", bufs=1, space="PSUM") as pw_pool,
            tc.tile_pool(name="dram", bufs=1, space="DRAM") as dpool,
        ):
            # --- persistent small tensors ---
            wkqw_sb = cpool.tile([128, DC * W33], bf16, tag="wkqw_sb")
            nc.sync.dma_start(out=wkqw_sb[:], in_=wkqw)
            if USE_CC:
                sel_sb = cpool.tile([IDX, 1], i32, tag="sel_sb")
                nc.sync.dma_start(out=sel_sb[:], in_=sel)

            # s33/qT/ksib padded to 128 partitions: score matmuls use
            # K=128 (zero-padded stationary) so the PE HAM activity
            # monitor sees a busy array and un-throttles 1.2 -> 2.4 GHz
            s33_sb = cpool.tile([128, T], bf16, tag="s33_sb")
            qT_sb = cpool.tile([128, T], bf16, tag="qT_sb")
            ksib_sb = cpool.tile([128, T], bf16, tag="ksib_sb")
            w_colb = cpool.tile([128, T // 128], bf16, tag="w_colb")
            w_col = cpool.tile([128, T // 128], f32, tag="w_col")

            for p0 in range(32, 128, 32):
                nc.vector.memset(s33_sb[p0:p0 + 32, :], 0)
            nc.vector.memset(qT_sb[:], 0)
            nc.vector.memset(ksib_sb[:], 0)

            # PE warm-up: chained matmuls on a zeroed tile keep the
            # array busy during the input load so HAM un-throttles
            if WARM_N:
                warm_sb = cpool.tile([128, 512], bf16, tag="warm_sb")
                nc.vector.memset(warm_sb[:], 0)
                pwarm = pw_pool.tile([128, 512], f32, tag="pwarm")
                for wi in range(WARM_N):
                    nc.tensor.matmul(
                        pwarm[:], warm_sb[:, 0:128], warm_sb[:],
                        start=(wi == 0), stop=(wi == WARM_N - 1),
                    )

            if USE_CC:
                kin = [dpool.tile([IDX, 1024], bf16, name=f"kin{st}",
                                  tag=f"kin{st}") for st in range(2)]
                kg = [dpool.tile([2 * IDX, 1024], bf16, name=f"kg{st}",
                                 tag=f"kg{st}") for st in range(2)]

            # --- input slabs (SP hwdge queue, first in stream);
            # host quad-layout gives 4 KB descriptors ---
            slabs = []
            for s in range(n_slabs):
                slab = slab_pool.tile([128, DC * 512], bf16, tag="slab")
                nc.sync.dma_start(
                    out=slab[:].rearrange("p (q n) -> p q n", q=4),
                    in_=xh[s * 512:(s + 1) * 512, :].rearrange(
                        "(q p) n -> p q n", q=4))
                slabs.append(slab)

            # --- projections per 512-token group ---
            for g in range(n_slabs):
                slab_v = slabs[g][:].rearrange("p (kd t) -> p kd t", kd=DC)
                pj = pj_pool.tile([W33, 512], f32, tag="pj")
                for kd in range(DC):
                    nc.tensor.matmul(
                        pj[:],
                        wkqw_sb[:, kd * W33:(kd + 1) * W33],
                        slab_v[:, kd, :],
                        start=(kd == 0), stop=(kd == DC - 1),
                    )
                c0, c1 = g * 512, (g + 1) * 512
                if g < NG:
                    nc.vector.tensor_copy(s33_sb[0:W33, c0:c1], pj[:])
                    # engine reads need 32-aligned partition offsets;
                    # DMAs don't — extract q rows and transposed w by DMA
                    nc.gpsimd.dma_start(
                        out=qT_sb[0:IDX, c0:c1],
                        in_=s33_sb[IDX:2 * IDX, c0:c1])
                    for gi in range(4):
                        t0 = c0 + gi * 128
                        nc.gpsimd.dma_start(
                            out=w_colb[:, g * 4 + gi:g * 4 + gi + 1],
                            in_=s33_sb[2 * IDX:W33, t0:t0 + 128],
                        )
                    nc.vector.tensor_copy(
                        w_col[:, g * 4:(g + 1) * 4],
                        w_colb[:, g * 4:(g + 1) * 4])
                    if USE_CC and g % 2 == 1:
                        st = g // 2
                        nc.gpsimd.dma_start(
                            out=kin[st][:],
                            in_=s33_sb[0:IDX, st * 1024:(st + 1) * 1024])
                        nc.gpsimd.collective_compute(
                            "AllGather",
                            mybir.AluOpType.bypass,
                            replica_groups=groups,
                            ins=[kin[st].opt()],
                            outs=[kg[st].opt()],
                        )
                else:
                    # fallback: other-half keys computed locally
                    nc.vector.tensor_copy(
                        ksib_sb[0:IDX, c0 - T:c1 - T], pj[0:IDX, :])

            if USE_CC:
                # sibling-slot select via per-core index gather
                for st in range(2):
                    nc.gpsimd.indirect_dma_start(
                        out=ksib_sb[0:IDX, st * 1024:(st + 1) * 1024],
                        out_offset=None,
                        in_=kg[st][:],
                        in_offset=bass.IndirectOffsetOnAxis(
                            ap=sel_sb[:, 0:1], axis=0),
                    )

            # --- scores + postproc + output ---
            # sub-phases: (krhs, rhs col base, o col base, n groups)
            # A scores own keys locally (no exchange wait); B0/B1 wait
            # on the two pipelined collectives
            TT = T // 128
            subphases = [
                (s33_sb, 0, 0, 2),          # A : own keys, 2x1024 cols
                (ksib_sb, 0, 2048, 1),      # B0: sibling first 1024
                (ksib_sb, 1024, 3072, 1),   # B1: sibling second 1024
            ]
            gidx = 0
            for krhs, rb, ob, ngrp in subphases:
                for i in range(TT):
                    osb = out_pool.tile([128, ngrp * 1024], bf16, tag="osb")
                    for cc in range(ngrp):
                        ps = ps_pool.tile([128, 1024], f32, tag="ps")
                        for jj in range(2):
                            j0 = rb + cc * 1024 + jj * 512
                            nc.tensor.matmul(
                                ps[:, jj * 512:(jj + 1) * 512],
                                qT_sb[:, i * 128:(i + 1) * 128],
                                krhs[:, j0:j0 + 512],
                                start=True, stop=True,
                            )
                        oslice = osb[:, cc * 1024:(cc + 1) * 1024]
                        gidx += 1
                        if FUSED and gidx % 4 == 3:
                            # fused relu+gate on DVE
                            nc.vector.tensor_scalar(
                                out=oslice,
                                in0=ps[:],
                                scalar1=0.0,
                                scalar2=w_col[:, i:i + 1],
                                op0=mybir.AluOpType.max,
                                op1=mybir.AluOpType.mult,
                            )
                        else:
                            nc.scalar.activation(
                                oslice, ps[:],
                                mybir.ActivationFunctionType.Relu,
                            )
                            nc.vector.tensor_scalar_mul(
                                out=oslice,
                                in0=oslice,
                                scalar1=w_col[:, i:i + 1],
                            )
                    nc.sync.dma_start(
                        out=o[i * 128:(i + 1) * 128, ob:ob + ngrp * 1024],
                        in_=osb[:],
                    )
    nc.compile()
    _CACHE["nc"] = nc
    return nc


def _make_in_maps(x, Wq, Wk, Ww):
    import ml_dtypes
    bf = ml_dtypes.bfloat16
    w33 = np.concatenate([Wk, Wq, Ww], axis=1).astype(bf)       # [D, 33]
    wkqw = np.ascontiguousarray(
        w33.reshape(DC, 128, W33).transpose(1, 0, 2).reshape(128, DC * W33))
    xbf = x.astype(bf)
    n_slabs = NG if USE_CC else 2 * NG
    in_maps = []
    for c in range(N_CORES):
        b, h = c // 2, c % 2
        own = xbf[b, h * T:(h + 1) * T, :]                       # [T, D]
        if USE_CC:
            xt = own.T                                            # [D, T]
        else:
            oth = xbf[b, (1 - h) * T:(2 - h) * T, :]
            xt = np.concatenate([own, oth], axis=0).T             # [D, S]
        ntok = xt.shape[1]
        # quad layout: row = s*512 + q*128 + p, col = (kd%4)*512 + t
        # (kd = q*4 + kd%4) so DMA descriptors are 4 KB
        xs = np.ascontiguousarray(
            xt.reshape(4, 4, 128, ntok // 512, 512)
            .transpose(3, 0, 2, 1, 4).reshape(n_slabs * 512, 2048))
        im = {"xh": xs, "wkqw": wkqw}
        if USE_CC:
            im["sel"] = ((1 - h) * IDX
                         + np.arange(IDX, dtype=np.int32)).reshape(IDX, 1)
        in_maps.append(im)
    return in_maps


def _assemble(results):
    out = np.empty((B, S, S), dtype=np.float32)
    for c in range(N_CORES):
        b, h = c // 2, c % 2
        oc = np.asarray(results[c]["o"], dtype=np.float32)
        r0 = h * T
        out[b, r0:r0 + T, h * T:(h + 1) * T] = oc[:, 0:T]
        out[b, r0:r0 + T, (1 - h) * T:(2 - h) * T] = oc[:, T:S]
    return out


def kernel(x, Wq, Wk, Ww, _trace_kwargs=None):
    nc = _build_nc()
    in_maps = _make_in_maps(np.asarray(x, dtype=np.float32),
                            np.asarray(Wq, dtype=np.float32),
                            np.asarray(Wk, dtype=np.float32),
                            np.asarray(Ww, dtype=np.float32))
    kw = _trace_kwargs or {}
    res = run_bass_kernel_spmd(nc, in_maps, list(range(N_CORES)), **kw)
    out = _assemble(res.results)
    if _trace_kwargs is not None:
        return out, res
    return out


# revision 73
# speedup vs baseline: 1.6196x; 1.6196x over previous
"""Trainium2 Bass kernel for MinimalLightningIndexer.

out[b,t,s] = relu((x@Wq)[b,t] . (x@Wk)[b,s]) * (x@Ww)[b,t]

Sharding: 8 cores = 4 batches x 2 token-halves. Each core receives ONLY
its own half of x[b] (transposed, 8.4 MB bf16) and computes [k|q|w]
projections for its 2048 tokens in one fused PE pass. Phase A scores the
2048 own queries against the 2048 LOCAL keys immediately; the 16-dim key
rows are exchanged with the sibling core in two pipelined pairwise
AllGathers (2 x 32 KB), and phases B0/B1 score against the sibling keys
(selected from the gather output by a per-core index-gather DMA, keeping
the program SPMD-uniform). Output [2048, 4096] bf16 = [own | sibling]
columns; the host reassembles with 2 block copies per core.

PE clock note: score matmuls contract only 16 idx dims; with K=16 the
HAM activity monitor reads the array as idle and keeps the PE throttled
at 1.2 GHz. The stationary q operand is therefore zero-padded to K=128
(full-array activity + fast weight load), and warm-up matmuls keep the
array busy during the input load.
"""

import os
import sys

if "/opt/trn_rl_repo" not in sys.path:
    sys.path.insert(0, "/opt/trn_rl_repo")

import numpy as np

import concourse.bacc as bacc
import concourse.bass as bass
import concourse.mybir as mybir
import concourse.tile as tile
from concourse.bass_utils import run_bass_kernel_spmd

B, S, D = 4, 4096, 2048
IDX = 16
N_CORES = 8
T = S // 2           # own tokens per core
DC = D // 128        # 16 d-chunks
NG = 4               # projection groups (512 tokens each)
W33 = 2 * IDX + 1    # [k | q | w] projection width

USE_CC = os.environ.get("K_USE_CC", "1") == "1"    # AllGather k-exchange
FUSED = os.environ.get("K_FUSED", "1") == "1"      # DVE fused relu+gate
WARM_N = int(os.environ.get("K_WARM_N", "56"))     # PE warm-up matmuls

_CACHE = {}


def _build_nc():
    if "nc" in _CACHE:
        return _CACHE["nc"]
    f32 = mybir.dt.float32
    bf16 = mybir.dt.bfloat16
    i32 = mybir.dt.int32
    nc = bacc.Bacc("TRN2", target_bir_lowering=False, debug=False,
                   num_devices=N_CORES)

    n_slabs = NG if USE_CC else 2 * NG
    xh = nc.dram_tensor("xh", [n_slabs * 128, DC * 512], bf16,
                        kind="ExternalInput").ap()
    wkqw = nc.dram_tensor("wkqw", [128, DC * W33], bf16,
                          kind="ExternalInput").ap()
    if USE_CC:
        sel = nc.dram_tensor("sel", [IDX, 1], i32, kind="ExternalInput").ap()
    o = nc.dram_tensor("o", [T, S], bf16, kind="ExternalOutput").ap()

    groups = [[2 * i, 2 * i + 1] for i in range(N_CORES // 2)]

    with tile.TileContext(nc) as tc:
        with (
            tc.tile_pool(name="const", bufs=1) as cpool,
            tc.tile_pool(name="slab", bufs=3) as slab_pool,
            tc.tile_pool(name="osb", bufs=8) as out_pool,
            tc.tile_pool(name="pj", bufs=2, space="PSUM") as pj_pool,
            tc.tile_pool(name="ps", bufs=2, space="PSUM") as ps_pool,
            tc.tile_pool(name="pw", bufs=1, space="PSUM") as pw_pool,
            tc.tile_pool(name="dram", bufs=1, space="DRAM") as dpool,
        ):
            # --- persistent small tensors ---
            wkqw_sb = cpool.tile([128, DC * W33], bf16, tag="wkqw_sb")
            nc.sync.dma_start(out=wkqw_sb[:], in_=wkqw)
            if USE_CC:
                sel_sb = cpool.tile([IDX, 1], i32, tag="sel_sb")
                nc.sync.dma_start(out=sel_sb[:], in_=sel)

            # s33/qT/ksib padded to 128 partitions: score matmuls use
            # K=128 (zero-padded stationary) so the PE HAM activity
            # monitor sees a busy array and un-throttles 1.2 -> 2.4 GHz
            s33_sb = cpool.tile([128, T], bf16, tag="s33_sb")
            qT_sb = cpool.tile([128, T], bf16, tag="qT_sb")
            ksib_sb = cpool.tile([128, T], bf16, tag="ksib_sb")
            w_colb = cpool.tile([128, T // 128], bf16, tag="w_colb")
            w_col = cpool.tile([128, T // 128], f32, tag="w_col")

            for p0 in range(32, 128, 32):
                nc.vector.memset(s33_sb[p0:p0 + 32, :], 0)
            nc.vector.memset(qT_sb[:], 0)
            nc.vector.memset(ksib_sb[:], 0)

            # PE warm-up: chained matmuls on a zeroed tile keep the
            # array busy during the input load so HAM un-throttles
            if WARM_N:
                warm_sb = cpool.tile([128, 512], bf16, tag="warm_sb")
                nc.vector.memset(warm_sb[:], 0)
                pwarm = pw_pool.tile([128, 512], f32, tag="pwarm")
                for wi in range(WARM_N):
                    nc.tensor.matmul(
                        pwarm[:], warm_sb[:, 0:128], warm_sb[:],
                        start=(wi == 0), stop=(wi == WARM_N - 1),
                    )

            if USE_CC:
                kin = [dpool.tile([IDX, 1024], bf16, name=f"kin{st}",
                                  tag=f"kin{st}") for st in range(2)]
                kg = [dpool.tile([2 * IDX, 1024], bf16, name=f"kg{st}",
                                 tag=f"kg{st}") for st in range(2)]

            # --- input slabs (SP hwdge queue, first in stream) ---
            slabs = []
            for s in range(n_slabs):
                slab = slab_pool.tile([128, DC * 512], bf16, tag="slab")
                nc.sync.dma_start(
                    out=slab[:], in_=xh[s * 128:(s + 1) * 128, :])
                slabs.append(slab)

            # --- projections per 512-token group ---
            for g in range(n_slabs):
                slab_v = slabs[g][:].rearrange("p (kd t) -> p kd t", kd=DC)
                pj = pj_pool.tile([W33, 512], f32, tag="pj")
                for kd in range(DC):
                    nc.tensor.matmul(
                        pj[:],
                        wkqw_sb[:, kd * W33:(kd + 1) * W33],
                        slab_v[:, kd, :],
                        start=(kd == 0), stop=(kd == DC - 1),
                    )
                c0, c1 = g * 512, (g + 1) * 512
                if g < NG:
                    nc.vector.tensor_copy(s33_sb[0:W33, c0:c1], pj[:])
                    # engine reads need 32-aligned partition offsets;
                    # DMAs don't — extract q rows and transposed w by DMA
                    nc.gpsimd.dma_start(
                        out=qT_sb[0:IDX, c0:c1],
                        in_=s33_sb[IDX:2 * IDX, c0:c1])
                    if USE_CC and g % 2 == 1:
                        # kin/collective first: they gate phase B, the
                        # w-column extraction below does not
                        st = g // 2
                        nc.gpsimd.dma_start(
                            out=kin[st][:],
                            in_=s33_sb[0:IDX, st * 1024:(st + 1) * 1024])
                        nc.gpsimd.collective_compute(
                            "AllGather",
                            mybir.AluOpType.bypass,
                            replica_groups=groups,
                            ins=[kin[st].opt()],
                            outs=[kg[st].opt()],
                        )
                    for gi in range(4):
                        t0 = c0 + gi * 128
                        nc.gpsimd.dma_start(
                            out=w_colb[:, g * 4 + gi:g * 4 + gi + 1],
                            in_=s33_sb[2 * IDX:W33, t0:t0 + 128],
                        )
                    nc.vector.tensor_copy(
                        w_col[:, g * 4:(g + 1) * 4],
                        w_colb[:, g * 4:(g + 1) * 4])
                else:
                    # fallback: other-half keys computed locally
                    nc.vector.tensor_copy(
                        ksib_sb[0:IDX, c0 - T:c1 - T], pj[0:IDX, :])

            if USE_CC:
                # sibling-slot select via per-core index gather
                for st in range(2):
                    nc.gpsimd.indirect_dma_start(
                        out=ksib_sb[0:IDX, st * 1024:(st + 1) * 1024],
                        out_offset=None,
                        in_=kg[st][:],
                        in_offset=bass.IndirectOffsetOnAxis(
                            ap=sel_sb[:, 0:1], axis=0),
                    )

            # --- scores + postproc + output ---
            # sub-phases: (krhs, rhs col base, o col base, n groups)
            # A scores own keys locally (no exchange wait); B0/B1 wait
            # on the two pipelined collectives
            TT = T // 128
            subphases = [
                (s33_sb, 0, 0, 2),          # A : own keys, 2x1024 cols
                (ksib_sb, 0, 2048, 1),      # B0: sibling first 1024
                (ksib_sb, 1024, 3072, 1),   # B1: sibling second 1024
            ]
            gidx = 0
            for krhs, rb, ob, ngrp in subphases:
                for i in range(TT):
                    osb = out_pool.tile([128, ngrp * 1024], bf16, tag="osb")
                    for cc in range(ngrp):
                        ps = ps_pool.tile([128, 1024], f32, tag="ps")
                        for jj in range(2):
                            j0 = rb + cc * 1024 + jj * 512
                            nc.tensor.matmul(
                                ps[:, jj * 512:(jj + 1) * 512],
                                qT_sb[:, i * 128:(i + 1) * 128],
                                krhs[:, j0:j0 + 512],
                                start=True, stop=True,
                            )
                        oslice = osb[:, cc * 1024:(cc + 1) * 1024]
                        gidx += 1
                        if FUSED and gidx % 4 == 3:
                            # fused relu+gate on DVE
                            nc.vector.tensor_scalar(
                                out=oslice,
                                in0=ps[:],
                                scalar1=0.0,
                                scalar2=w_col[:, i:i + 1],
                                op0=mybir.AluOpType.max,
                                op1=mybir.AluOpType.mult,
                            )
                        else:
                            nc.scalar.activation(
                                oslice, ps[:],
                                mybir.ActivationFunctionType.Relu,
                            )
                            nc.vector.tensor_scalar_mul(
                                out=oslice,
                                in0=oslice,
                                scalar1=w_col[:, i:i + 1],
                            )
                    nc.sync.dma_start(
                        out=o[i * 128:(i + 1) * 128, ob:ob + ngrp * 1024],
                        in_=osb[:],
                    )
    nc.compile()
    _CACHE["nc"] = nc
    return nc


def _make_in_maps(x, Wq, Wk, Ww):
    import ml_dtypes
    bf = ml_dtypes.bfloat16
    w33 = np.concatenate([Wk, Wq, Ww], axis=1).astype(bf)       # [D, 33]
    wkqw = np.ascontiguousarray(
        w33.reshape(DC, 128, W33).transpose(1, 0, 2).reshape(128, DC * W33))
    xbf = x.astype(bf)
    n_slabs = NG if USE_CC else 2 * NG
    in_maps = []
    for c in range(N_CORES):
        b, h = c // 2, c % 2
        own = xbf[b, h * T:(h + 1) * T, :]                       # [T, D]
        if USE_CC:
            xt = own.T                                            # [D, T]
        else:
            oth = xbf[b, (1 - h) * T:(2 - h) * T, :]
            xt = np.concatenate([own, oth], axis=0).T             # [D, S]
        ntok = xt.shape[1]
        xs = np.ascontiguousarray(
            xt.reshape(DC, 128, ntok // 512, 512)
            .transpose(2, 1, 0, 3).reshape(n_slabs * 128, DC * 512))
        im = {"xh": xs, "wkqw": wkqw}
        if USE_CC:
            im["sel"] = ((1 - h) * IDX
                         + np.arange(IDX, dtype=np.int32)).reshape(IDX, 1)
        in_maps.append(im)
    return in_maps


def _assemble(results):
    out = np.empty((B, S, S), dtype=np.float32)
    for c in range(N_CORES):
        b, h = c // 2, c % 2
        oc = np.asarray(results[c]["o"], dtype=np.float32)
        r0 = h * T
        out[b, r0:r0 + T, h * T:(h + 1) * T] = oc[:, 0:T]
        out[b, r0:r0 + T, (1 - h) * T:(2 - h) * T] = oc[:, T:S]
    return out


def kernel(x, Wq, Wk, Ww, _trace_kwargs=None):
    nc = _build_nc()
    in_maps = _make_in_maps(np.asarray(x, dtype=np.float32),
                            np.asarray(Wq, dtype=np.float32),
                            np.asarray(Wk, dtype=np.float32),
                            np.asarray(Ww, dtype=np.float32))
    kw = _trace_kwargs or {}
    res = run_bass_kernel_spmd(nc, in_maps, list(range(N_CORES)), **kw)
    out = _assemble(res.results)
    if _trace_kwargs is not None:
        return out, res
    return out


# revision 75
# speedup vs baseline: 1.7019x; 1.0508x over previous
"""Trainium2 Bass kernel for MinimalLightningIndexer.

out[b,t,s] = relu((x@Wq)[b,t] . (x@Wk)[b,s]) * (x@Ww)[b,t]

Sharding: 8 cores = 4 batches x 2 token-halves. Each core receives ONLY
its own half of x[b] (transposed, 8.4 MB bf16) and computes [k|q|w]
projections for its 2048 tokens in one fused PE pass. Phase A scores the
2048 own queries against the 2048 LOCAL keys immediately; the 16-dim key
rows are exchanged with the sibling core in two pipelined pairwise
AllGathers (2 x 32 KB), and phases B0/B1 score against the sibling keys
(selected from the gather output by a per-core index-gather DMA, keeping
the program SPMD-uniform). Output [2048, 4096] bf16 = [own | sibling]
columns; the host reassembles with 2 block copies per core.

PE clock note: score matmuls contract only 16 idx dims; with K=16 the
HAM activity monitor reads the array as idle and keeps the PE throttled
at 1.2 GHz. The stationary q operand is therefore zero-padded to K=128
(full-array activity + fast weight load), and warm-up matmuls keep the
array busy during the input load.
"""

import os
import sys

if "/opt/trn_rl_repo" not in sys.path:
    sys.path.insert(0, "/opt/trn_rl_repo")

import numpy as np

import concourse.bacc as bacc
import concourse.bass as bass
import concourse.mybir as mybir
import concourse.tile as tile
from concourse.bass_utils import run_bass_kernel_spmd

B, S, D = 4, 4096, 2048
IDX = 16
N_CORES = 8
T = S // 2           # own tokens per core
DC = D // 128        # 16 d-chunks
NG = 4               # projection groups (512 tokens each)
W33 = 2 * IDX + 1    # [k | q | w] projection width

USE_CC = os.environ.get("K_USE_CC", "1") == "1"    # AllGather k-exchange
FUSED = os.environ.get("K_FUSED", "1") == "1"      # DVE fused relu+gate
WARM_N = int(os.environ.get("K_WARM_N", "56"))     # PE warm-up matmuls

_CACHE = {}


def _build_nc():
    if "nc" in _CACHE:
        return _CACHE["nc"]
    f32 = mybir.dt.float32
    bf16 = mybir.dt.bfloat16
    i32 = mybir.dt.int32
    nc = bacc.Bacc("TRN2", target_bir_lowering=False, debug=False,
                   num_devices=N_CORES)

    n_slabs = NG if USE_CC else 2 * NG
    xh = nc.dram_tensor("xh", [n_slabs * 128, DC * 512], bf16,
                        kind="ExternalInput").ap()
    wkqw = nc.dram_tensor("wkqw", [128, DC * W33], bf16,
                          kind="ExternalInput").ap()
    if USE_CC:
        sel = nc.dram_tensor("sel", [IDX, 1], i32, kind="ExternalInput").ap()
    o = nc.dram_tensor("o", [T, S], bf16, kind="ExternalOutput").ap()

    groups = [[2 * i, 2 * i + 1] for i in range(N_CORES // 2)]

    with tile.TileContext(nc) as tc:
        with (
            tc.tile_pool(name="const", bufs=1) as cpool,
            tc.tile_pool(name="slab", bufs=3) as slab_pool,
            tc.tile_pool(name="osb", bufs=8) as out_pool,
            tc.tile_pool(name="pj", bufs=2, space="PSUM") as pj_pool,
            tc.tile_pool(name="ps", bufs=2, space="PSUM") as ps_pool,
            tc.tile_pool(name="pw", bufs=1, space="PSUM") as pw_pool,
            tc.tile_pool(name="dram", bufs=1, space="DRAM") as dpool,
        ):
            # PE warm-up emitted first: chained matmuls on a zeroed
            # tile keep the array busy during the input load so HAM
            # un-throttles; its memset leads the DVE stream
            if WARM_N:
                warm_sb = cpool.tile([128, 512], bf16, tag="warm_sb")
                nc.vector.memset(warm_sb[:], 0)
                pwarm = pw_pool.tile([128, 512], f32, tag="pwarm")
                for wi in range(WARM_N):
                    nc.tensor.matmul(
                        pwarm[:], warm_sb[:, 0:128], warm_sb[:],
                        start=(wi == 0), stop=(wi == WARM_N - 1),
                    )

            # --- persistent small tensors (ACT hwdge queue, so the
            # sync queue leads with the input slabs) ---
            wkqw_sb = cpool.tile([128, DC * W33], bf16, tag="wkqw_sb")
            nc.scalar.dma_start(out=wkqw_sb[:], in_=wkqw)
            if USE_CC:
                sel_sb = cpool.tile([IDX, 1], i32, tag="sel_sb")
                nc.scalar.dma_start(out=sel_sb[:], in_=sel)

            # s33/qT/ksib padded to 128 partitions: score matmuls use
            # K=128 (zero-padded stationary) so the PE HAM activity
            # monitor sees a busy array and un-throttles 1.2 -> 2.4 GHz
            s33_sb = cpool.tile([128, T], bf16, tag="s33_sb")
            qT_sb = cpool.tile([128, T], bf16, tag="qT_sb")
            ksib_sb = cpool.tile([128, T], bf16, tag="ksib_sb")
            w_colb = cpool.tile([128, T // 128], bf16, tag="w_colb")
            w_col = cpool.tile([128, T // 128], f32, tag="w_col")

            for p0 in range(32, 128, 32):
                nc.vector.memset(s33_sb[p0:p0 + 32, :], 0)
            nc.vector.memset(qT_sb[:], 0)
            nc.vector.memset(ksib_sb[:], 0)

            if USE_CC:
                kin = [dpool.tile([IDX, 1024], bf16, name=f"kin{st}",
                                  tag=f"kin{st}") for st in range(2)]
                kg = [dpool.tile([2 * IDX, 1024], bf16, name=f"kg{st}",
                                 tag=f"kg{st}") for st in range(2)]

            # --- input slabs (SP hwdge queue, first in stream) ---
            slabs = []
            for s in range(n_slabs):
                slab = slab_pool.tile([128, DC * 512], bf16, tag="slab")
                nc.sync.dma_start(
                    out=slab[:], in_=xh[s * 128:(s + 1) * 128, :])
                slabs.append(slab)

            # --- projections per 512-token group ---
            for g in range(n_slabs):
                slab_v = slabs[g][:].rearrange("p (kd t) -> p kd t", kd=DC)
                pj = pj_pool.tile([W33, 512], f32, tag="pj")
                for kd in range(DC):
                    nc.tensor.matmul(
                        pj[:],
                        wkqw_sb[:, kd * W33:(kd + 1) * W33],
                        slab_v[:, kd, :],
                        start=(kd == 0), stop=(kd == DC - 1),
                    )
                c0, c1 = g * 512, (g + 1) * 512
                if g < NG:
                    nc.vector.tensor_copy(s33_sb[0:W33, c0:c1], pj[:])
                    # engine reads need 32-aligned partition offsets;
                    # DMAs don't — extract q rows and transposed w by DMA
                    nc.gpsimd.dma_start(
                        out=qT_sb[0:IDX, c0:c1],
                        in_=s33_sb[IDX:2 * IDX, c0:c1])
                    if USE_CC and g % 2 == 1:
                        # kin/collective first: they gate phase B, the
                        # w-column extraction below does not
                        st = g // 2
                        nc.gpsimd.dma_start(
                            out=kin[st][:],
                            in_=s33_sb[0:IDX, st * 1024:(st + 1) * 1024])
                        nc.gpsimd.collective_compute(
                            "AllGather",
                            mybir.AluOpType.bypass,
                            replica_groups=groups,
                            ins=[kin[st].opt()],
                            outs=[kg[st].opt()],
                        )
                    for gi in range(4):
                        t0 = c0 + gi * 128
                        nc.gpsimd.dma_start(
                            out=w_colb[:, g * 4 + gi:g * 4 + gi + 1],
                            in_=s33_sb[2 * IDX:W33, t0:t0 + 128],
                        )
                    nc.vector.tensor_copy(
                        w_col[:, g * 4:(g + 1) * 4],
                        w_colb[:, g * 4:(g + 1) * 4])
                else:
                    # fallback: other-half keys computed locally
                    nc.vector.tensor_copy(
                        ksib_sb[0:IDX, c0 - T:c1 - T], pj[0:IDX, :])

            if USE_CC:
                # sibling-slot select via per-core index gather
                for st in range(2):
                    nc.gpsimd.indirect_dma_start(
                        out=ksib_sb[0:IDX, st * 1024:(st + 1) * 1024],
                        out_offset=None,
                        in_=kg[st][:],
                        in_offset=bass.IndirectOffsetOnAxis(
                            ap=sel_sb[:, 0:1], axis=0),
                    )

            # --- scores + postproc + output ---
            # sub-phases: (krhs, rhs col base, o col base, n groups)
            # A scores own keys locally (no exchange wait); B0/B1 wait
            # on the two pipelined collectives
            TT = T // 128
            subphases = [
                (s33_sb, 0, 0, 2),          # A : own keys, 2x1024 cols
                (ksib_sb, 0, 2048, 1),      # B0: sibling first 1024
                (ksib_sb, 1024, 3072, 1),   # B1: sibling second 1024
            ]
            gidx = 0
            for krhs, rb, ob, ngrp in subphases:
                for i in range(TT):
                    osb = out_pool.tile([128, ngrp * 1024], bf16, tag="osb")
                    for cc in range(ngrp):
                        ps = ps_pool.tile([128, 1024], f32, tag="ps")
                        for jj in range(2):
                            j0 = rb + cc * 1024 + jj * 512
                            nc.tensor.matmul(
                                ps[:, jj * 512:(jj + 1) * 512],
                                qT_sb[:, i * 128:(i + 1) * 128],
                                krhs[:, j0:j0 + 512],
                                start=True, stop=True,
                            )
                        oslice = osb[:, cc * 1024:(cc + 1) * 1024]
                        gidx += 1
                        if FUSED and gidx % 4 == 3:
                            # fused relu+gate on DVE
                            nc.vector.tensor_scalar(
                                out=oslice,
                                in0=ps[:],
                                scalar1=0.0,
                                scalar2=w_col[:, i:i + 1],
                                op0=mybir.AluOpType.max,
                                op1=mybir.AluOpType.mult,
                            )
                        else:
                            nc.scalar.activation(
                                oslice, ps[:],
                                mybir.ActivationFunctionType.Relu,
                            )
                            nc.vector.tensor_scalar_mul(
                                out=oslice,
                                in0=oslice,
                                scalar1=w_col[:, i:i + 1],
                            )
                    nc.sync.dma_start(
                        out=o[i * 128:(i + 1) * 128, ob:ob + ngrp * 1024],
                        in_=osb[:],
                    )
    nc.compile()
    _CACHE["nc"] = nc
    return nc


def _make_in_maps(x, Wq, Wk, Ww):
    import ml_dtypes
    bf = ml_dtypes.bfloat16
    w33 = np.concatenate([Wk, Wq, Ww], axis=1).astype(bf)       # [D, 33]
    wkqw = np.ascontiguousarray(
        w33.reshape(DC, 128, W33).transpose(1, 0, 2).reshape(128, DC * W33))
    xbf = x.astype(bf)
    n_slabs = NG if USE_CC else 2 * NG
    in_maps = []
    for c in range(N_CORES):
        b, h = c // 2, c % 2
        own = xbf[b, h * T:(h + 1) * T, :]                       # [T, D]
        if USE_CC:
            xt = own.T                                            # [D, T]
        else:
            oth = xbf[b, (1 - h) * T:(2 - h) * T, :]
            xt = np.concatenate([own, oth], axis=0).T             # [D, S]
        ntok = xt.shape[1]
        xs = np.ascontiguousarray(
            xt.reshape(DC, 128, ntok // 512, 512)
            .transpose(2, 1, 0, 3).reshape(n_slabs * 128, DC * 512))
        im = {"xh": xs, "wkqw": wkqw}
        if USE_CC:
            im["sel"] = ((1 - h) * IDX
                         + np.arange(IDX, dtype=np.int32)).reshape(IDX, 1)
        in_maps.append(im)
    return in_maps


def _assemble(results):
    out = np.empty((B, S, S), dtype=np.float32)
    for c in range(N_CORES):
        b, h = c // 2, c % 2
        oc = np.asarray(results[c]["o"], dtype=np.float32)
        r0 = h * T
        out[b, r0:r0 + T, h * T:(h + 1) * T] = oc[:, 0:T]
        out[b, r0:r0 + T, (1 - h) * T:(2 - h) * T] = oc[:, T:S]
    return out


def kernel(x, Wq, Wk, Ww, _trace_kwargs=None):
    nc = _build_nc()
    in_maps = _make_in_maps(np.asarray(x, dtype=np.float32),
                            np.asarray(Wq, dtype=np.float32),
                            np.asarray(Wk, dtype=np.float32),
                            np.asarray(Ww, dtype=np.float32))
    kw = _trace_kwargs or {}
    res = run_bass_kernel_spmd(nc, in_maps, list(range(N_CORES)), **kw)
    out = _assemble(res.results)
    if _trace_kwargs is not None:
        return out, res
    return out
